# revision 13
# baseline (speedup 1.0000x reference)
"""BFFN (linear-attention style gated FFN) Trainium2 Bass kernel, 8 NeuronCores.

Reference computation (all fp32, B=4, N=4096, D=E=1024):
    query = (x_real @ Wqr) * (x_imag @ Wqi)        # [b, n, e]
    key   = x_real @ Wk                             # [b, n, d]
    value = x_imag @ Wv                             # [b, n, e]
    kv    = einsum('bnd,bne->bde', key, value)      # [b, d, e]
    out   = einsum('bnd,bde->bne', query, kv)       # [b, n, e]

Algebraic restructure: kv = Wk^T @ (xr^T @ xi) @ Wv.  With S = xr^T @ xi
(the only sequence-length reduction), the kv path costs N*D*D + 2*D*D*E
instead of 3*N*D*E FLOPs, and S is computed from x in NATURAL layout.

Sharding: 8 cores = 4 batches x 2 sequence-halves.  Each pair AllReduces
its partial S (bf16, as two pipelined 1MB halves); each core then computes
its dk-HALF of kv (via a host-sliced wk input: even core gets wk cols
0:512, odd core 512:1024 -- the program stays uniform) and the pair
AllGathers the kv halves (concat lands in global dk order).

Schedule (v5).  Trace-driven facts this version is built on: the PE
p-state halves after an idle gap; a HW DMA ring stalls at a gated head
descriptor and is ordered by the scheduler's readiness model (late-ready
descriptors sink behind ready-at-0 bulk); a collective doorbell
transitively waits on everything ahead of its staging in ring order; an
un-throttled ring round-robins packets across all queued descriptors
(first tile ~13us; throttling the ring destroys its bandwidth).  Hence:
  - x^T is transposed ON THE HOST and streamed pre-packed (no PE
    transposes); x tiles 0-1 are split into quarter-piece DMAs so the
    first S matmul starts ~8us earlier at full queue depth.
  - all 8 PSUM banks accumulate S f-half0 during the stream; f-half1
    replays from resident x.
  - collective staging INPUTS ride the gpsimd software ring (always
    empty, and the doorbell lives on gpsimd anyway) so AR0's doorbell
    fires right after the drains instead of behind the weight streams.
  - collective-output loads (s_sb halves, kv_sb) ride the HW rings where
    readiness-ordering naturally places them behind the bulk; x^T chunks
    2-3 ride the sync ring so their slot-reuse gating can never delay
    the scalar ring's s_sb f-half1 load (the 10.6us UT stall in v3).
  - PE order: S fh0 (stream) -> fh1 replay -> query chunks 0-1 (cover
    the AllReduces) -> UT -> kv + both AllGathers -> query chunks 2-3
    (cover the gathers) -> out.
All matmuls bf16 operands, fp32 PSUM accumulation.
"""
import numpy as np

import concourse.bass as bass
import concourse.mybir as mybir
import concourse.tile as tile
from concourse import bacc
from concourse.bass import ts, ds
from concourse.bass_utils import run_bass_kernel_spmd

F32 = mybir.dt.float32
BF16 = mybir.dt.bfloat16

B, N, D, E = 4, 4096, 1024, 1024
N_CORES = 8
NL = N // 2          # 2048 rows (sequence) per core
P = 128
NT = NL // P         # 16 n-tiles
DT = D // P          # 8 d tiles
ET = E // P          # 8 e tiles
FD = 512             # matmul moving free dim / PSUM bank
NCH = NL // FD       # 4 n-chunks of 512
HK = 512             # dk half owned per core

REPLICA_GROUPS = [[0, 1], [2, 3], [4, 5], [6, 7]]


def build_bass():
    nc = bacc.Bacc("TRN2", target_bir_lowering=False, debug=False,
                   num_devices=N_CORES)

    xr = nc.dram_tensor("xr", [NL, D], BF16, kind="ExternalInput").ap()
    xi = nc.dram_tensor("xi", [NL, D], BF16, kind="ExternalInput").ap()
    # host-transposed x, pre-packed so chunk c is one contiguous [P, DT*FD]
    # blob: row (c*P+p), col (t*FD+f)  =  x[c*FD+f, t*P+p]
    xrt = nc.dram_tensor("xrt", [NCH * P, DT * FD], BF16,
                         kind="ExternalInput").ap()
    xit = nc.dram_tensor("xit", [NCH * P, DT * FD], BF16,
                         kind="ExternalInput").ap()
    # wq in host-packed e-halves: query chunk 0's first et groups gate on
    # half 0 only, landing ~7us earlier than a single whole-wq DMA
    wqr_h = [nc.dram_tensor(f"wqr{h}", [D, HK], BF16,
                            kind="ExternalInput").ap() for h in range(2)]
    wqi_h = [nc.dram_tensor(f"wqi{h}", [D, HK], BF16,
                            kind="ExternalInput").ap() for h in range(2)]
    wk = nc.dram_tensor("wk", [D, HK], BF16, kind="ExternalInput").ap()
    wv = nc.dram_tensor("wv", [D, E], BF16, kind="ExternalInput").ap()
    out = nc.dram_tensor("out", [NL, E], BF16, kind="ExternalOutput").ap()

    def as_tiles(w):  # [1024, n] DRAM view -> [128, 8, n] partition-major
        return w.rearrange("(t p) n -> p t n", p=P)

    with tile.TileContext(nc) as tc:
        with (
            tc.tile_pool(name="big", bufs=3) as big_pool,      # x/qt/s/kv
            tc.tile_pool(name="xtc", bufs=4) as xtc_pool,      # xT chunk ring
            tc.tile_pool(name="wp", bufs=1) as w_pool,
            tc.tile_pool(name="sst", bufs=2) as sst_pool,      # staging ring
            tc.tile_pool(name="prst", bufs=3) as prt_pool,
            tc.tile_pool(name="outst", bufs=4) as out_pool,
            tc.tile_pool(name="ps", bufs=1, space="PSUM") as ps_pool,
            tc.tile_pool(name="dram", bufs=1, space="DRAM") as dram_pool,
        ):
            bnc_s_in = [dram_pool.tile([D, FD], BF16, tag=f"si{h}",
                                       name=f"bnc_s_in{h}") for h in range(2)]
            bnc_s_out = [dram_pool.tile([D, FD], BF16, tag=f"so{h}",
                                        name=f"bnc_s_out{h}") for h in range(2)]
            bnc_kv_in = [dram_pool.tile([HK, FD], BF16, tag=f"ki{h}",
                                        name=f"bnc_kv_in{h}") for h in range(2)]
            bnc_kv_out = [dram_pool.tile([D, FD], BF16, tag=f"ko{h}",
                                         name=f"bnc_kv_out{h}") for h in range(2)]

            wqr_sb = w_pool.tile([P, DT, E], BF16, tag="wqr", name="wqr_sb")
            wqi_sb = w_pool.tile([P, DT, E], BF16, tag="wqi", name="wqi_sb")
            wk_sb = w_pool.tile([P, DT, HK], BF16, tag="wk", name="wk_sb")
            wv_sb = w_pool.tile([P, DT, E], BF16, tag="wv", name="wv_sb")

            xr_nat = big_pool.tile([P, NT, D], BF16, tag="big", name="xr_nat")
            xi_nat = big_pool.tile([P, NT, D], BF16, tag="big", name="xi_nat")
            qt_sb = big_pool.tile([P, ET, NL], BF16, tag="big", name="qt_sb")

            def drain(dst, src, k):
                # alternate PSUM-drain engines so the copies never form a
                # serial chain on the DVE
                if k % 2:
                    nc.scalar.copy(dst, src)
                else:
                    nc.vector.tensor_copy(dst, src)

            # ---- stream phase: x natural on both HW rings; all 8 PSUM
            # banks accumulate S f-half0 per arriving tile.  Tiles 0-1 are
            # posted as quarter-pieces: under full-queue round-robin each
            # descriptor gets ~1/16 of ring bandwidth, so smaller first
            # descriptors complete proportionally sooner. ----
            ps_s0 = [ps_pool.tile([P, FD], F32, tag="ps8", bufs=8,
                                  name=f"ps_s0_{k}") for k in range(DT)]
            for nt in range(NT):
                nc.sync.dma_start(xr_nat[:, nt, :], xr[ts(nt, P), :])
                nc.scalar.dma_start(xi_nat[:, nt, :], xi[ts(nt, P), :])
                if nt >= 3 and nt % 2 == 1:
                    # engine-side nop that WAITS on tile nt-3 before later
                    # posts: caps the ring at ~4 outstanding tile
                    # descriptors, so the first tiles are not diluted by
                    # packet round-robin over the whole queue (first S
                    # matmul starts ~6us earlier); unlike an in-ring dummy
                    # DMA this keeps the ring itself free to run.
                    for eng, xnat in ((nc.sync, xr_nat), (nc.scalar, xi_nat)):
                        nop = eng.nop(hint="dep").ins
                        nop.ins = [eng.lower_ap(xnat[0:1, nt - 3, 0:1])]
                for k in range(DT):
                    nc.tensor.matmul(
                        ps_s0[k][:], xr_nat[:, nt, ts(k, P)],
                        xi_nat[:, nt, :FD],
                        start=(nt == 0), stop=(nt == NT - 1),
                    )

            # fh0 drain -> staging on the gpsimd SOFTWARE ring (empty, and
            # the AllReduce doorbell is a gpsimd instruction anyway) so the
            # doorbell fires right after the drains
            s_st0 = sst_pool.tile([P, DT, FD], BF16, tag="sst", name="s_st0")
            # S-phase drains go ONLY to the vector engine: a scalar-engine
            # drain here gets scheduled BEHIND the scalar ring's bulk
            # descriptor posts (which block ~20us on ring backpressure),
            # stalling the fh1 replay and the AR0 staging behind it.
            for k in range(DT):
                nc.vector.tensor_copy(s_st0[:, k, :], ps_s0[k][:])
            for k in range(DT):
                nc.gpsimd.dma_start(as_tiles(bnc_s_in[0])[:, k, :],
                                    s_st0[:, k, :])
            nc.gpsimd.collective_compute(
                "AllReduce", mybir.AluOpType.add,
                replica_groups=REPLICA_GROUPS,
                ins=[bnc_s_in[0].opt()], outs=[bnc_s_out[0].opt()],
            )

            # bulk streams on the HW rings (they sequence behind x there)
            xt_c = {}

            def post_chunk(c, r_eng, i_eng):
                xt_c[c] = (
                    xtc_pool.tile([P, DT, FD], BF16, tag="xtc",
                                  name=f"xt_r{c}"),
                    xtc_pool.tile([P, DT, FD], BF16, tag="xtc",
                                  name=f"xt_i{c}"),
                )
                r_eng.dma_start(xt_c[c][0][:], xrt[ts(c, P), :])
                i_eng.dma_start(xt_c[c][1][:], xit[ts(c, P), :])

            # qc0's gate is max(c0, wq): c0 first, then wq halves in order
            post_chunk(0, nc.sync, nc.scalar)
            for h in range(2):
                nc.sync.dma_start(wqr_sb[:, :, ts(h, FD)], as_tiles(wqr_h[h]))
                nc.scalar.dma_start(wqi_sb[:, :, ts(h, FD)],
                                    as_tiles(wqi_h[h]))
            post_chunk(1, nc.sync, nc.scalar)
            nc.sync.dma_start(wk_sb[:], as_tiles(wk))
            nc.scalar.dma_start(wv_sb[:], as_tiles(wv))

            # ---- S f-half1 replay from resident x -> AllReduce1 ----
            ps_s1 = [ps_pool.tile([P, FD], F32, tag="ps8", bufs=8,
                                  name=f"ps_s1_{k}") for k in range(DT)]
            for nt in range(NT):
                for k in range(DT):
                    nc.tensor.matmul(
                        ps_s1[k][:], xr_nat[:, nt, ts(k, P)],
                        xi_nat[:, nt, FD:],
                        start=(nt == 0), stop=(nt == NT - 1),
                    )
            s_st1 = sst_pool.tile([P, DT, FD], BF16, tag="sst", name="s_st1")
            for k in range(DT):     # vector-only: see s_st0 comment
                nc.vector.tensor_copy(s_st1[:, k, :], ps_s1[k][:])
            for k in range(DT):
                nc.gpsimd.dma_start(as_tiles(bnc_s_in[1])[:, k, :],
                                    s_st1[:, k, :])
            nc.gpsimd.collective_compute(
                "AllReduce", mybir.AluOpType.add,
                replica_groups=REPLICA_GROUPS,
                ins=[bnc_s_in[1].opt()], outs=[bnc_s_out[1].opt()],
            )

            # reduced S: recycle xr_nat's slot (dead after the fh1 replay).
            # fh0 half on sync, fh1 half on scalar -- each ring's readiness
            # ordering puts the load right after the corresponding AR.
            s_sb = big_pool.tile([P, DT, D], BF16, tag="big", name="s_sb")
            nc.sync.dma_start(s_sb[:, :, :FD], as_tiles(bnc_s_out[0]))
            nc.scalar.dma_start(s_sb[:, :, FD:], as_tiles(bnc_s_out[1]))

            def query_chunk(c, xtr_c, xti_c):
                for et in range(ET):
                    ps_r = ps_pool.tile([P, FD], F32, tag="ps8", bufs=8,
                                        name="ps_qr")
                    for d in range(DT):
                        nc.tensor.matmul(
                            ps_r[:], wqr_sb[:, d, ts(et, P)], xtr_c[:, d, :],
                            start=(d == 0), stop=(d == DT - 1),
                        )
                    prt = prt_pool.tile([P, FD], BF16, tag="prt", name="prt")
                    nc.vector.tensor_copy(prt[:], ps_r[:])
                    ps_i = ps_pool.tile([P, FD], F32, tag="ps8", bufs=8,
                                        name="ps_qi")
                    for d in range(DT):
                        nc.tensor.matmul(
                            ps_i[:], wqi_sb[:, d, ts(et, P)], xti_c[:, d, :],
                            start=(d == 0), stop=(d == DT - 1),
                        )
                    nc.vector.tensor_mul(
                        out=qt_sb[:, et, ts(c, FD)], in0=prt[:], in1=ps_i[:],
                    )

            # ---- query chunks 0-2 cover the AllReduce latency (qc2 before
            # UT absorbs run-to-run AllReduce duration variance) ----
            query_chunk(0, *xt_c[0])

            # xT chunks 2-3 BOTH on the sync ring: their ring-slot reuse
            # gates on qc0/qc1, and on the scalar ring that gating would
            # delay the s_sb f-half1 load behind them (v3's 10.6us UT stall)
            post_chunk(2, nc.sync, nc.sync)
            query_chunk(1, *xt_c[1])
            post_chunk(3, nc.sync, nc.sync)
            query_chunk(2, *xt_c[2])

            # ---- UT = S^T wk_half: [f 1024, dk-own 512] ----
            # f-half0 tiles gate on AR0 only, f-half1 on AR1
            ut_sb = sst_pool.tile([P, DT, HK], BF16, tag="sst", name="ut_sb")
            for dpt in range(DT):      # f-tile of UT's partition dim
                ps_u = ps_pool.tile([P, HK], F32, tag="ps8", bufs=8,
                                    name="ps_u")
                for d in range(DT):
                    nc.tensor.matmul(
                        ps_u[:], s_sb[:, d, ts(dpt, P)], wk_sb[:, d, :],
                        start=(d == 0), stop=(d == DT - 1),
                    )
                drain(ut_sb[:, dpt, :], ps_u[:], dpt)

            # ---- kv_own = UT^T wv -> pipelined AllGathers (staging on the
            # gpsimd software ring, doorbells right behind it) ----
            kv_st = sst_pool.tile([P, 4, E], BF16, tag="sst", name="kv_st")
            for eh in range(2):
                for dkt in range(4):   # local dk tile
                    ps_k = ps_pool.tile([P, FD], F32, tag="ps8", bufs=8,
                                        name="ps_k")
                    for dp in range(DT):
                        nc.tensor.matmul(
                            ps_k[:], ut_sb[:, dp, ts(dkt, P)],
                            wv_sb[:, dp, ts(eh, FD)],
                            start=(dp == 0), stop=(dp == DT - 1),
                        )
                    drain(kv_st[:, dkt, ts(eh, FD)], ps_k[:], dkt)
                for dkt in range(4):
                    nc.gpsimd.dma_start(
                        bnc_kv_in[eh].rearrange("(t p) n -> p t n",
                                                p=P)[:, dkt, :],
                        kv_st[:, dkt, ts(eh, FD)])
                nc.gpsimd.collective_compute(
                    "AllGather", mybir.AluOpType.bypass,
                    replica_groups=REPLICA_GROUPS,
                    ins=[bnc_kv_in[eh].opt()], outs=[bnc_kv_out[eh].opt()],
                )

            # full kv in global dk order: recycle xi_nat's slot; both halves
            # on scalar (ready late -> readiness ordering parks them last)
            kv_sb = big_pool.tile([P, DT, E], BF16, tag="big", name="kv_sb")
            for eh in range(2):
                nc.scalar.dma_start(kv_sb[:, :, ts(eh, FD)],
                                    as_tiles(bnc_kv_out[eh]))

            # ---- query chunk 3 covers the AllGathers ----
            query_chunk(3, *xt_c[3])

            # ---- out = queryT.T @ kv: eh0 sweep then eh1 sweep, so only
            # the eh1 groups gate on the second AllGather ----
            for eh in range(2):
                for nt in range(NT):
                    ps_o = ps_pool.tile([P, FD], F32, tag="ps8", bufs=8,
                                        name="ps_o")
                    for et in range(ET):
                        nc.tensor.matmul(
                            ps_o[:], qt_sb[:, et, ts(nt, P)],
                            kv_sb[:, et, ts(eh, FD)],
                            start=(et == 0), stop=(et == ET - 1),
                        )
                    o_st = out_pool.tile([P, FD], BF16, tag="ost", name="o_st")
                    drain(o_st[:], ps_o[:], nt)
                    # 3-ring round-robin keeps any single ring's credit
                    # window from backpressuring the final writes
                    eng = (nc.sync, nc.scalar, nc.gpsimd)[(2 * nt + eh) % 3]
                    eng.dma_start(out[ts(nt, P), ts(eh, FD)], o_st[:])

    nc.compile()
    return nc


def make_in_maps(x_real, x_imag, w_query_real, w_query_imag, w_key, w_value):
    import ml_dtypes
    bf16 = ml_dtypes.bfloat16

    def cast(a):  # host-side bf16 cast: identical to the on-device DVE cast
        return np.ascontiguousarray(np.asarray(a, dtype=np.float32)
                                    .astype(bf16))

    def pack_t(x_half):
        # [NL, D] -> x^T pre-packed per chunk: out[c*P+p, t*FD+f]
        #   = x[c*FD+f, t*P+p]
        return np.ascontiguousarray(
            x_half.reshape(NCH, FD, DT, P).transpose(0, 3, 2, 1)
            .reshape(NCH * P, DT * FD))

    ws = {
        "wqr0": cast(w_query_real[:, :HK]),
        "wqr1": cast(w_query_real[:, HK:]),
        "wqi0": cast(w_query_imag[:, :HK]),
        "wqi1": cast(w_query_imag[:, HK:]),
        "wv": cast(w_value),
    }
    wk_halves = [cast(w_key[:, h * HK:(h + 1) * HK]) for h in range(2)]
    in_maps = []
    for c in range(N_CORES):
        b, h = divmod(c, 2)
        sl = slice(h * NL, (h + 1) * NL)
        xr_h = cast(x_real[b, sl])
        xi_h = cast(x_imag[b, sl])
        in_maps.append({
            "xr": xr_h,
            "xi": xi_h,
            "xrt": pack_t(xr_h),
            "xit": pack_t(xi_h),
            "wk": wk_halves[h],
            **ws,
        })
    return in_maps


def gather_out(results):
    out = np.empty((B, N, E), np.float32)
    for c in range(N_CORES):
        b, h = divmod(c, 2)
        out[b, h * NL:(h + 1) * NL] = np.asarray(results[c]["out"],
                                                 dtype=np.float32)
    return out


def kernel(x_real, x_imag, w_query_real, w_query_imag, w_key, w_value):
    nc = build_bass()
    in_maps = make_in_maps(x_real, x_imag, w_query_real, w_query_imag,
                           w_key, w_value)
    res = run_bass_kernel_spmd(nc, in_maps, core_ids=list(range(N_CORES)))
    return gather_out(res.results)


if __name__ == "__main__":
    rng = np.random.default_rng(0)
    args = dict(
        x_real=rng.standard_normal((B, N, D), dtype=np.float32),
        x_imag=rng.standard_normal((B, N, D), dtype=np.float32),
        w_query_real=(rng.standard_normal((D, E), dtype=np.float32) / D),
        w_query_imag=(rng.standard_normal((D, E), dtype=np.float32) / D),
        w_key=(rng.standard_normal((D, E), dtype=np.float32) / D),
        w_value=(rng.standard_normal((D, E), dtype=np.float32) / D),
    )
    got = kernel(**args)
    q = np.einsum("bnd,de->bne", args["x_real"], args["w_query_real"]) * \
        np.einsum("bnd,de->bne", args["x_imag"], args["w_query_imag"])
    k = np.einsum("bnd,de->bne", args["x_real"], args["w_key"])
    v = np.einsum("bnd,de->bne", args["x_imag"], args["w_value"])
    kv = np.einsum("bnd,bne->bde", k, v)
    want = np.einsum("bnd,bde->bne", q, kv)
    denom = np.abs(want).max()
    print("max abs err:", np.abs(got - want).max())
    print("rel err:", np.abs(got - want).max() / denom)


# revision 15
# speedup vs baseline: 1.0101x; 1.0101x over previous
"""BFFN (linear-attention style gated FFN) Trainium2 Bass kernel, 8 NeuronCores.

Reference computation (all fp32, B=4, N=4096, D=E=1024):
    query = (x_real @ Wqr) * (x_imag @ Wqi)        # [b, n, e]
    key   = x_real @ Wk                             # [b, n, d]
    value = x_imag @ Wv                             # [b, n, e]
    kv    = einsum('bnd,bne->bde', key, value)      # [b, d, e]
    out   = einsum('bnd,bde->bne', query, kv)       # [b, n, e]

Algebraic restructure: kv = Wk^T @ (xr^T @ xi) @ Wv.  With S = xr^T @ xi
(the only sequence-length reduction), the kv path costs N*D*D + 2*D*D*E
instead of 3*N*D*E FLOPs, and S is computed from x in NATURAL layout.

Sharding: 8 cores = 4 batches x 2 sequence-halves.  Each pair AllReduces
its partial S (bf16, as two pipelined 1MB halves); each core then computes
its dk-HALF of kv (via a host-sliced wk input: even core gets wk cols
0:512, odd core 512:1024 -- the program stays uniform) and the pair
AllGathers the kv halves (concat lands in global dk order).

Schedule (v8, 268us vs the 335us v2 baseline).  Trace-driven facts this
version is built on: the PE p-state halves after an idle gap (3us
re-ramp), so the schedule is built to keep the PE literally gap-free; a
HW DMA ring stalls at a gated head descriptor and the Tile scheduler
orders ring content by its readiness model (late-ready descriptors sink
behind ready-at-0 bulk); a collective doorbell transitively waits on
everything ahead of its staging in ring order; an un-throttled ring
round-robins packets across ALL queued descriptors (first x tile lands
~13us late at full queue depth, but gating descriptors inside the ring
destroys its bandwidth -- throttle from the ENGINE side instead); an
engine-side PSUM drain scheduled behind a bulk descriptor post can block
~20us on ring backpressure.  Hence:
  - x^T is transposed ON THE HOST and streamed pre-packed, one
    contiguous 1MB blob per chunk (no PE transposes at all).
  - x tile posting is throttled by engine-side NOPs that wait on tile
    nt-3 (~4 tiles outstanding): the first S matmul starts at ~11us
    instead of ~25us, with no ring-bandwidth loss.
  - all 8 PSUM banks accumulate S f-half0 during the stream; f-half1
    replays from resident x.  S-phase drains are VECTOR-ONLY so they
    can never be scheduled behind the scalar engine's blocked posts.
  - collective staging INPUTS ride the gpsimd software ring (always
    empty; the doorbell lives on gpsimd anyway) so AR0 fires ~54us in.
  - collective-output loads (s_sb halves, kv_sb) ride the HW rings
    where readiness-ordering parks them behind the bulk; x^T chunks
    2-3 ride the sync ring so their slot-reuse gating can never delay
    the scalar ring's s_sb f-half1 load.
  - wq streams as host-packed e-halves behind x^T chunk 0, so query
    chunk 0 starts the moment the fh1 replay retires.
  - PE order: S fh0 (stream) -> fh1 replay -> query chunks 0-2 (cover
    both AllReduces with ~30us of margin for their run-to-run variance)
    -> UT -> kv + both AllGathers -> query chunk 3 (covers the gathers)
    -> out.  The result: PE dense from 11us to the last matmul.
All matmuls bf16 operands, fp32 PSUM accumulation.
"""
import numpy as np

import concourse.bass as bass
import concourse.mybir as mybir
import concourse.tile as tile
from concourse import bacc
from concourse.bass import ts, ds
from concourse.bass_utils import run_bass_kernel_spmd

F32 = mybir.dt.float32
BF16 = mybir.dt.bfloat16

B, N, D, E = 4, 4096, 1024, 1024
N_CORES = 8
NL = N // 2          # 2048 rows (sequence) per core
P = 128
NT = NL // P         # 16 n-tiles
DT = D // P          # 8 d tiles
ET = E // P          # 8 e tiles
FD = 512             # matmul moving free dim / PSUM bank
NCH = NL // FD       # 4 n-chunks of 512
HK = 512             # dk half owned per core

REPLICA_GROUPS = [[0, 1], [2, 3], [4, 5], [6, 7]]


def build_bass():
    nc = bacc.Bacc("TRN2", target_bir_lowering=False, debug=False,
                   num_devices=N_CORES)

    xr = nc.dram_tensor("xr", [NL, D], BF16, kind="ExternalInput").ap()
    xi = nc.dram_tensor("xi", [NL, D], BF16, kind="ExternalInput").ap()
    # host-transposed x, pre-packed so chunk c is one contiguous [P, DT*FD]
    # blob: row (c*P+p), col (t*FD+f)  =  x[c*FD+f, t*P+p]
    xrt = nc.dram_tensor("xrt", [NCH * P, DT * FD], BF16,
                         kind="ExternalInput").ap()
    xit = nc.dram_tensor("xit", [NCH * P, DT * FD], BF16,
                         kind="ExternalInput").ap()
    # wq in host-packed e-halves: query chunk 0's first et groups gate on
    # half 0 only, landing ~7us earlier than a single whole-wq DMA
    wqr_h = [nc.dram_tensor(f"wqr{h}", [D, HK], BF16,
                            kind="ExternalInput").ap() for h in range(2)]
    wqi_h = [nc.dram_tensor(f"wqi{h}", [D, HK], BF16,
                            kind="ExternalInput").ap() for h in range(2)]
    wk = nc.dram_tensor("wk", [D, HK], BF16, kind="ExternalInput").ap()
    wv = nc.dram_tensor("wv", [D, E], BF16, kind="ExternalInput").ap()
    out = nc.dram_tensor("out", [NL, E], BF16, kind="ExternalOutput").ap()

    def as_tiles(w):  # [1024, n] DRAM view -> [128, 8, n] partition-major
        return w.rearrange("(t p) n -> p t n", p=P)

    with tile.TileContext(nc) as tc:
        with (
            tc.tile_pool(name="big", bufs=3) as big_pool,      # x/qt/s/kv
            tc.tile_pool(name="xtc", bufs=4) as xtc_pool,      # xT chunk ring
            tc.tile_pool(name="wp", bufs=1) as w_pool,
            tc.tile_pool(name="sst", bufs=2) as sst_pool,      # staging ring
            tc.tile_pool(name="prst", bufs=3) as prt_pool,
            tc.tile_pool(name="outst", bufs=4) as out_pool,
            tc.tile_pool(name="ps", bufs=1, space="PSUM") as ps_pool,
            tc.tile_pool(name="dram", bufs=1, space="DRAM") as dram_pool,
        ):
            bnc_s_in = [dram_pool.tile([D, FD], BF16, tag=f"si{h}",
                                       name=f"bnc_s_in{h}") for h in range(2)]
            bnc_s_out = [dram_pool.tile([D, FD], BF16, tag=f"so{h}",
                                        name=f"bnc_s_out{h}") for h in range(2)]
            bnc_kv_in = [dram_pool.tile([HK, FD], BF16, tag=f"ki{h}",
                                        name=f"bnc_kv_in{h}") for h in range(2)]
            bnc_kv_out = [dram_pool.tile([D, FD], BF16, tag=f"ko{h}",
                                         name=f"bnc_kv_out{h}") for h in range(2)]

            wqr_sb = w_pool.tile([P, DT, E], BF16, tag="wqr", name="wqr_sb")
            wqi_sb = w_pool.tile([P, DT, E], BF16, tag="wqi", name="wqi_sb")
            wk_sb = w_pool.tile([P, DT, HK], BF16, tag="wk", name="wk_sb")
            wv_sb = w_pool.tile([P, DT, E], BF16, tag="wv", name="wv_sb")

            xr_nat = big_pool.tile([P, NT, D], BF16, tag="big", name="xr_nat")
            xi_nat = big_pool.tile([P, NT, D], BF16, tag="big", name="xi_nat")
            qt_sb = big_pool.tile([P, ET, NL], BF16, tag="big", name="qt_sb")

            def drain(dst, src, k):
                # alternate PSUM-drain engines so the copies never form a
                # serial chain on the DVE
                if k % 2:
                    nc.scalar.copy(dst, src)
                else:
                    nc.vector.tensor_copy(dst, src)

            # ---- stream phase: x natural on both HW rings; all 8 PSUM
            # banks accumulate S f-half0 per arriving tile ----
            ps_s0 = [ps_pool.tile([P, FD], F32, tag="ps8", bufs=8,
                                  name=f"ps_s0_{k}") for k in range(DT)]
            for nt in range(NT):
                nc.sync.dma_start(xr_nat[:, nt, :], xr[ts(nt, P), :])
                nc.scalar.dma_start(xi_nat[:, nt, :], xi[ts(nt, P), :])
                if nt >= 3 and nt % 2 == 1:
                    # engine-side nop that WAITS on tile nt-3 before later
                    # posts: caps the ring at ~4 outstanding tile
                    # descriptors, so the first tiles are not diluted by
                    # packet round-robin over the whole queue (first S
                    # matmul starts ~6us earlier); unlike an in-ring dummy
                    # DMA this keeps the ring itself free to run.
                    for eng, xnat in ((nc.sync, xr_nat), (nc.scalar, xi_nat)):
                        nop = eng.nop(hint="dep").ins
                        nop.ins = [eng.lower_ap(xnat[0:1, nt - 3, 0:1])]
                for k in range(DT):
                    nc.tensor.matmul(
                        ps_s0[k][:], xr_nat[:, nt, ts(k, P)],
                        xi_nat[:, nt, :FD],
                        start=(nt == 0), stop=(nt == NT - 1),
                    )

            # fh0 drain -> staging on the gpsimd SOFTWARE ring (empty, and
            # the AllReduce doorbell is a gpsimd instruction anyway) so the
            # doorbell fires right after the drains
            s_st0 = sst_pool.tile([P, DT, FD], BF16, tag="sst", name="s_st0")
            # S-phase drains go ONLY to the vector engine: a scalar-engine
            # drain here gets scheduled BEHIND the scalar ring's bulk
            # descriptor posts (which block ~20us on ring backpressure),
            # stalling the fh1 replay and the AR0 staging behind it.
            for k in range(DT):
                nc.vector.tensor_copy(s_st0[:, k, :], ps_s0[k][:])
            for k in range(DT):
                nc.gpsimd.dma_start(as_tiles(bnc_s_in[0])[:, k, :],
                                    s_st0[:, k, :])
            nc.gpsimd.collective_compute(
                "AllReduce", mybir.AluOpType.add,
                replica_groups=REPLICA_GROUPS,
                ins=[bnc_s_in[0].opt()], outs=[bnc_s_out[0].opt()],
            )

            # bulk streams on the HW rings (they sequence behind x there)
            xt_c = {}

            def post_chunk(c, r_eng, i_eng):
                xt_c[c] = (
                    xtc_pool.tile([P, DT, FD], BF16, tag="xtc",
                                  name=f"xt_r{c}"),
                    xtc_pool.tile([P, DT, FD], BF16, tag="xtc",
                                  name=f"xt_i{c}"),
                )
                r_eng.dma_start(xt_c[c][0][:], xrt[ts(c, P), :])
                i_eng.dma_start(xt_c[c][1][:], xit[ts(c, P), :])

            # qc0's gate is max(c0, wq): c0 first, then wq halves in order
            post_chunk(0, nc.sync, nc.scalar)
            for h in range(2):
                nc.sync.dma_start(wqr_sb[:, :, ts(h, FD)], as_tiles(wqr_h[h]))
                nc.scalar.dma_start(wqi_sb[:, :, ts(h, FD)],
                                    as_tiles(wqi_h[h]))
            post_chunk(1, nc.sync, nc.scalar)
            nc.sync.dma_start(wk_sb[:], as_tiles(wk))
            nc.scalar.dma_start(wv_sb[:], as_tiles(wv))

            # ---- S f-half1 replay from resident x -> AllReduce1 ----
            ps_s1 = [ps_pool.tile([P, FD], F32, tag="ps8", bufs=8,
                                  name=f"ps_s1_{k}") for k in range(DT)]
            for nt in range(NT):
                for k in range(DT):
                    nc.tensor.matmul(
                        ps_s1[k][:], xr_nat[:, nt, ts(k, P)],
                        xi_nat[:, nt, FD:],
                        start=(nt == 0), stop=(nt == NT - 1),
                    )
            s_st1 = sst_pool.tile([P, DT, FD], BF16, tag="sst", name="s_st1")
            for k in range(DT):     # vector-only: see s_st0 comment
                nc.vector.tensor_copy(s_st1[:, k, :], ps_s1[k][:])
            for k in range(DT):
                nc.gpsimd.dma_start(as_tiles(bnc_s_in[1])[:, k, :],
                                    s_st1[:, k, :])
            nc.gpsimd.collective_compute(
                "AllReduce", mybir.AluOpType.add,
                replica_groups=REPLICA_GROUPS,
                ins=[bnc_s_in[1].opt()], outs=[bnc_s_out[1].opt()],
            )

            # reduced S: recycle xr_nat's slot (dead after the fh1 replay).
            # fh0 half on sync, fh1 half on scalar -- each ring's readiness
            # ordering puts the load right after the corresponding AR.
            s_sb = big_pool.tile([P, DT, D], BF16, tag="big", name="s_sb")
            nc.sync.dma_start(s_sb[:, :, :FD], as_tiles(bnc_s_out[0]))
            nc.scalar.dma_start(s_sb[:, :, FD:], as_tiles(bnc_s_out[1]))

            def query_chunk(c, xtr_c, xti_c):
                for et in range(ET):
                    ps_r = ps_pool.tile([P, FD], F32, tag="ps8", bufs=8,
                                        name="ps_qr")
                    for d in range(DT):
                        nc.tensor.matmul(
                            ps_r[:], wqr_sb[:, d, ts(et, P)], xtr_c[:, d, :],
                            start=(d == 0), stop=(d == DT - 1),
                        )
                    prt = prt_pool.tile([P, FD], BF16, tag="prt", name="prt")
                    nc.vector.tensor_copy(prt[:], ps_r[:])
                    ps_i = ps_pool.tile([P, FD], F32, tag="ps8", bufs=8,
                                        name="ps_qi")
                    for d in range(DT):
                        nc.tensor.matmul(
                            ps_i[:], wqi_sb[:, d, ts(et, P)], xti_c[:, d, :],
                            start=(d == 0), stop=(d == DT - 1),
                        )
                    nc.vector.tensor_mul(
                        out=qt_sb[:, et, ts(c, FD)], in0=prt[:], in1=ps_i[:],
                    )

            # ---- query chunks 0-2 cover the AllReduce latency (qc2 before
            # UT absorbs run-to-run AllReduce duration variance) ----
            query_chunk(0, *xt_c[0])

            # xT chunks 2-3 BOTH on the sync ring: their ring-slot reuse
            # gates on qc0/qc1, and on the scalar ring that gating would
            # delay the s_sb f-half1 load behind them (v3's 10.6us UT stall)
            post_chunk(2, nc.sync, nc.sync)
            query_chunk(1, *xt_c[1])
            post_chunk(3, nc.sync, nc.sync)
            query_chunk(2, *xt_c[2])

            # ---- UT = S^T wk_half: [f 1024, dk-own 512] ----
            # f-half0 tiles gate on AR0 only, f-half1 on AR1
            ut_sb = sst_pool.tile([P, DT, HK], BF16, tag="sst", name="ut_sb")
            for dpt in range(DT):      # f-tile of UT's partition dim
                ps_u = ps_pool.tile([P, HK], F32, tag="ps8", bufs=8,
                                    name="ps_u")
                for d in range(DT):
                    nc.tensor.matmul(
                        ps_u[:], s_sb[:, d, ts(dpt, P)], wk_sb[:, d, :],
                        start=(d == 0), stop=(d == DT - 1),
                    )
                drain(ut_sb[:, dpt, :], ps_u[:], dpt)

            # ---- kv_own = UT^T wv -> pipelined AllGathers (staging on the
            # gpsimd software ring, doorbells right behind it) ----
            kv_st = sst_pool.tile([P, 4, E], BF16, tag="sst", name="kv_st")
            for eh in range(2):
                for dkt in range(4):   # local dk tile
                    ps_k = ps_pool.tile([P, FD], F32, tag="ps8", bufs=8,
                                        name="ps_k")
                    for dp in range(DT):
                        nc.tensor.matmul(
                            ps_k[:], ut_sb[:, dp, ts(dkt, P)],
                            wv_sb[:, dp, ts(eh, FD)],
                            start=(dp == 0), stop=(dp == DT - 1),
                        )
                    drain(kv_st[:, dkt, ts(eh, FD)], ps_k[:], dkt)
                for dkt in range(4):
                    nc.gpsimd.dma_start(
                        bnc_kv_in[eh].rearrange("(t p) n -> p t n",
                                                p=P)[:, dkt, :],
                        kv_st[:, dkt, ts(eh, FD)])
                nc.gpsimd.collective_compute(
                    "AllGather", mybir.AluOpType.bypass,
                    replica_groups=REPLICA_GROUPS,
                    ins=[bnc_kv_in[eh].opt()], outs=[bnc_kv_out[eh].opt()],
                )

            # full kv in global dk order: recycle xi_nat's slot; both halves
            # on scalar (ready late -> readiness ordering parks them last)
            kv_sb = big_pool.tile([P, DT, E], BF16, tag="big", name="kv_sb")
            for eh in range(2):
                nc.scalar.dma_start(kv_sb[:, :, ts(eh, FD)],
                                    as_tiles(bnc_kv_out[eh]))

            # ---- query chunk 3 covers the AllGathers ----
            query_chunk(3, *xt_c[3])

            # ---- out = queryT.T @ kv: eh0 sweep then eh1 sweep, so only
            # the eh1 groups gate on the second AllGather ----
            for eh in range(2):
                for nt in range(NT):
                    ps_o = ps_pool.tile([P, FD], F32, tag="ps8", bufs=8,
                                        name="ps_o")
                    for et in range(ET):
                        nc.tensor.matmul(
                            ps_o[:], qt_sb[:, et, ts(nt, P)],
                            kv_sb[:, et, ts(eh, FD)],
                            start=(et == 0), stop=(et == ET - 1),
                        )
                    o_st = out_pool.tile([P, FD], BF16, tag="ost", name="o_st")
                    drain(o_st[:], ps_o[:], nt)
                    # 3-ring round-robin keeps any single ring's credit
                    # window from backpressuring the final writes
                    eng = (nc.sync, nc.scalar, nc.gpsimd)[(2 * nt + eh) % 3]
                    eng.dma_start(out[ts(nt, P), ts(eh, FD)], o_st[:])

    nc.compile()
    return nc


def make_in_maps(x_real, x_imag, w_query_real, w_query_imag, w_key, w_value):
    import ml_dtypes
    bf16 = ml_dtypes.bfloat16

    def cast(a):  # host-side bf16 cast: identical to the on-device DVE cast
        return np.ascontiguousarray(np.asarray(a, dtype=np.float32)
                                    .astype(bf16))

    def pack_t(x_half):
        # [NL, D] -> x^T pre-packed per chunk: out[c*P+p, t*FD+f]
        #   = x[c*FD+f, t*P+p]
        return np.ascontiguousarray(
            x_half.reshape(NCH, FD, DT, P).transpose(0, 3, 2, 1)
            .reshape(NCH * P, DT * FD))

    ws = {
        "wqr0": cast(w_query_real[:, :HK]),
        "wqr1": cast(w_query_real[:, HK:]),
        "wqi0": cast(w_query_imag[:, :HK]),
        "wqi1": cast(w_query_imag[:, HK:]),
        "wv": cast(w_value),
    }
    wk_halves = [cast(w_key[:, h * HK:(h + 1) * HK]) for h in range(2)]
    in_maps = []
    for c in range(N_CORES):
        b, h = divmod(c, 2)
        sl = slice(h * NL, (h + 1) * NL)
        xr_h = cast(x_real[b, sl])
        xi_h = cast(x_imag[b, sl])
        in_maps.append({
            "xr": xr_h,
            "xi": xi_h,
            "xrt": pack_t(xr_h),
            "xit": pack_t(xi_h),
            "wk": wk_halves[h],
            **ws,
        })
    return in_maps


def gather_out(results):
    out = np.empty((B, N, E), np.float32)
    for c in range(N_CORES):
        b, h = divmod(c, 2)
        out[b, h * NL:(h + 1) * NL] = np.asarray(results[c]["out"],
                                                 dtype=np.float32)
    return out


def kernel(x_real, x_imag, w_query_real, w_query_imag, w_key, w_value):
    nc = build_bass()
    in_maps = make_in_maps(x_real, x_imag, w_query_real, w_query_imag,
                           w_key, w_value)
    res = run_bass_kernel_spmd(nc, in_maps, core_ids=list(range(N_CORES)))
    return gather_out(res.results)


if __name__ == "__main__":
    rng = np.random.default_rng(0)
    args = dict(
        x_real=rng.standard_normal((B, N, D), dtype=np.float32),
        x_imag=rng.standard_normal((B, N, D), dtype=np.float32),
        w_query_real=(rng.standard_normal((D, E), dtype=np.float32) / D),
        w_query_imag=(rng.standard_normal((D, E), dtype=np.float32) / D),
        w_key=(rng.standard_normal((D, E), dtype=np.float32) / D),
        w_value=(rng.standard_normal((D, E), dtype=np.float32) / D),
    )
    got = kernel(**args)
    q = np.einsum("bnd,de->bne", args["x_real"], args["w_query_real"]) * \
        np.einsum("bnd,de->bne", args["x_imag"], args["w_query_imag"])
    k = np.einsum("bnd,de->bne", args["x_real"], args["w_key"])
    v = np.einsum("bnd,de->bne", args["x_imag"], args["w_value"])
    kv = np.einsum("bnd,bne->bde", k, v)
    want = np.einsum("bnd,bde->bne", q, kv)
    denom = np.abs(want).max()
    print("max abs err:", np.abs(got - want).max())
    print("rel err:", np.abs(got - want).max() / denom)
